# revision 3
# baseline (speedup 1.0000x reference)
"""Multi-head attention (B=2, S=2048, E=1024, H=16) on 8 Trainium2 cores.

Sharding: core c -> (batch b = c//4, head-group g = c%4 of 4 heads).
Each core computes Q/K/V projections for its 4 heads (256 features),
full attention for those heads, and a partial output projection
(256 rows of Wo). Host sums the 4 partials per batch element and adds bo.

Schedule (v3): built for engine overlap.
  - x DMAs ride TWO queues (sync+scalar, alternating feature tiles) in
    priority order xk, xq, xv; weights ride the gpsimd queue. y output
    rides sync/gpsimd (NOT scalar: the Scalar engine is the exp
    bottleneck and DMA triggers would steal ~0.7us each from it).
  - Lead-in: K-proj (both pairs) tracks xk arrival f-inner on 8 psum
    accumulators, then Q-proj (both pairs) tracks xq, then V-proj in two
    8-bank passes tracks xv. Attention starts as soon as Q/K are done;
    the first query-block's P@V chases the arriving V tiles through a
    deep ptt run-ahead buffer.
  - Attention is software-pipelined one key-tile deep: PE emits
    scores(kt), then PV(kt-1), so the exp(kt) on the Scalar engine
    overlaps PE work instead of stalling it (which also lets the PE
    p-state ramp to 2.4 GHz).
  - The loop is query-block-major (qb outer, pair inner) so the output
    projection (which needs both pairs' attention output for a token
    range) spreads uniformly over the whole phase instead of piling
    into pair 1.
  - Each block's normalization chain is DEFERRED into the next block's
    kt loop so the in-order PE never waits on the DVE denominator copy.

On-chip layouts (per core):
  qt/kt: (128 feat-part, pair, 2048 tok)  transposed proj outputs; the
         128 partitions hold two heads (64+64) per pair index.
  v:     (128 tok-part, 16 tok-tiles, 4*65): per head 64 dims plus a
         "ones" column produced by an augmented V projection (extra
         output feature with zero weights and bias 1.0); P @ V_aug then
         also yields the softmax denominator row for free.
  scores are computed transposed (key-pos on partitions, query on free)
  so exp runs on ACT along the free dim and P tiles feed P@V directly as
  the moving operand; no transposes anywhere in the pipeline.
"""

import numpy as np

B, S, E, H = 2, 2048, 1024, 16
D = 64
NCORES = 8
FPC = 256  # features (head dims) per core = 4 heads
VW = 4 * 65  # V-projection output width incl. ones columns

MODE = "bf16"

_PROGRAMS = {}
LAST_RESULT = None
TRACE = False
TRACE_DIR = None


def _build(mode):
    import concourse.tile as tile
    from concourse import bacc, mybir

    f32 = mybir.dt.float32
    DT = mybir.dt.bfloat16
    NW = 512
    NNB = S // NW  # 4 query blocks per pair

    nc = bacc.Bacc("TRN2", target_bir_lowering=False, debug=False,
                   num_devices=NCORES)

    xq_ap = nc.dram_tensor("xq", [E, S], DT, kind="ExternalInput").ap()
    xk_ap = nc.dram_tensor("xk", [E, S], DT, kind="ExternalInput").ap()
    xv_ap = nc.dram_tensor("xv", [E, S], DT, kind="ExternalInput").ap()
    wq_ap = nc.dram_tensor("wq", [128, 8, FPC], DT, kind="ExternalInput").ap()
    wk_ap = nc.dram_tensor("wk", [128, 8, FPC], DT, kind="ExternalInput").ap()
    wv_ap = nc.dram_tensor("wv", [128, 8, VW], DT, kind="ExternalInput").ap()
    wo_ap = nc.dram_tensor("wo", [128, 2, E], DT, kind="ExternalInput").ap()
    bqk_ap = nc.dram_tensor("bqk", [128, 4], f32, kind="ExternalInput").ap()
    bv_ap = nc.dram_tensor("bv", [1, VW], DT, kind="ExternalInput").ap()
    ones_ap = nc.dram_tensor("ones", [1, 128], DT, kind="ExternalInput").ap()
    y_ap = nc.dram_tensor("y", [S, E], f32, kind="ExternalOutput").ap()

    Exp = mybir.ActivationFunctionType.Exp

    with tile.TileContext(nc) as tc:
        with tc.tile_pool(name="persist", bufs=1) as persist:
            wq_sb = persist.tile([128, 8, FPC], DT, name="wq_sb")
            wk_sb = persist.tile([128, 8, FPC], DT, name="wk_sb")
            wv_sb = persist.tile([128, 8, VW], DT, name="wv_sb")
            wo_sb = persist.tile([128, 2, E], DT, name="wo_sb")
            bqk_sb = persist.tile([128, 4], f32, name="bqk_sb")
            bv_sb = persist.tile([1, VW], DT, name="bv_sb")
            ones_sb = persist.tile([1, 128], DT, name="ones_sb")
            qt_sb = persist.tile([128, 2, S], DT, name="qt_sb")
            kt_sb = persist.tile([128, 2, S], DT, name="kt_sb")
            v_sb = persist.tile([128, 16, VW], DT, name="v_sb")
            at_sb = persist.tile([128, 2, S], DT, name="at_sb")
            xq_res = persist.tile([128, 8, S], DT, name="xq_res")
            xk_res = persist.tile([128, 8, S], DT, name="xk_res")
            xv_res = persist.tile([128, 8, S], DT, name="xv_res")
            ones32 = ones_sb[:, 0:64]

            # weights/consts on the gpsimd DGE queue, k first (needed first)
            nc.gpsimd.dma_start(wk_sb, wk_ap)
            nc.gpsimd.dma_start(bqk_sb, bqk_ap)
            nc.gpsimd.dma_start(wq_sb, wq_ap)
            nc.gpsimd.dma_start(wv_sb, wv_ap)
            nc.gpsimd.dma_start(bv_sb, bv_ap)
            nc.gpsimd.dma_start(ones_sb, ones_ap)
            nc.gpsimd.dma_start(wo_sb, wo_ap)
            # x on TWO queues, alternating tiles, priority xk > xq > xv
            for xres, xap in ((xk_res, xk_ap), (xq_res, xq_ap),
                              (xv_res, xv_ap)):
                for f in range(8):
                    eng = nc.sync if f % 2 == 0 else nc.scalar
                    eng.dma_start(xres[:, f, :],
                                  xap[f * 128:(f + 1) * 128, :])

            def emit_qk_proj(pool, w_sb, x_res, out_sb, bcol):
                """f-inner projection, both pairs: 8 psum accumulators
                track the x feature tiles as they arrive."""
                pj = {}
                for p in range(2):
                    for nb in range(NNB):
                        pj[(p, nb)] = pool.tile(
                            [128, NW], f32, tag="proj", bufs=8,
                            name=f"pj_{bcol}_{p}_{nb}")
                for f in range(8):
                    for p in range(2):
                        for nb in range(NNB):
                            nc.tensor.matmul(
                                pj[(p, nb)],
                                w_sb[:, f, p * 128:(p + 1) * 128],
                                x_res[:, f, nb * NW:(nb + 1) * NW],
                                start=(f == 0), stop=(f == 7))
                for p in range(2):
                    for nb in range(NNB):
                        nc.vector.tensor_scalar_add(
                            out_sb[:, p, nb * NW:(nb + 1) * NW], pj[(p, nb)],
                            bqk_sb[:, bcol + p:bcol + p + 1])

            def emit_v_half(pool, half):
                """V projection for 8 token tiles, f-inner (tracks xv)."""
                psv = [pool.tile([128, VW], f32, tag="proj", bufs=8,
                                 name=f"pjv_{half}_{i}")
                       for i in range(8)]
                for i in range(8):
                    nc.tensor.matmul(psv[i], ones_sb, bv_sb,
                                     start=True, stop=False)
                for f in range(8):
                    for i in range(8):
                        tt = half * 8 + i
                        nc.tensor.matmul(
                            psv[i],
                            xv_res[:, f, tt * 128:(tt + 1) * 128],
                            wv_sb[:, f, :],
                            start=False, stop=(f == 7))
                for i in range(8):
                    nc.vector.tensor_copy(v_sb[:, half * 8 + i, :], psv[i])

            # ---- lead-in: K-proj, Q-proj (both pairs), V-proj ----
            with tc.tile_pool(name="lead", bufs=1, space="PSUM") as lead:
                emit_qk_proj(lead, wk_sb, xk_res, kt_sb, 2)
                emit_qk_proj(lead, wq_sb, xq_res, qt_sb, 0)
                emit_v_half(lead, 0)
                emit_v_half(lead, 1)

            # ---- attention, qb-major with deferred normalization ----
            with tc.tile_pool(name="pt", bufs=14) as ptpool, \
                 tc.tile_pool(name="sm", bufs=2) as smpool, \
                 tc.tile_pool(name="ysb", bufs=2) as ypool, \
                 tc.tile_pool(name="scps", bufs=2, space="PSUM") as scps, \
                 tc.tile_pool(name="pvps", bufs=2, space="PSUM") as pvps, \
                 tc.tile_pool(name="miscps", bufs=1, space="PSUM") as miscps:

                def make_norm(p, qb, pvt, last):
                    qsl = slice(qb * NW, (qb + 1) * NW)

                    def norm():
                        for hh in range(2):
                            denr = smpool.tile([1, NW], DT, tag="denr",
                                               name=f"dn_{p}_{qb}_{hh}")
                            nc.vector.tensor_copy(denr, pvt[hh][64:65, :])
                            rb = pvps.tile([64, NW], f32, tag="rb", bufs=1,
                                           name=f"rb_{p}_{qb}_{hh}")
                            nc.tensor.matmul(rb, ones32, denr,
                                             start=True, stop=True)
                            rbs = smpool.tile([64, NW], f32, tag="rbs",
                                              name=f"rbs_{p}_{qb}_{hh}")
                            nc.vector.reciprocal_approx_fast(rbs, rb)
                            nc.vector.tensor_mul(
                                at_sb[64 * hh:64 * hh + 64, p, qsl],
                                pvt[hh][0:64, :], rbs)
                        if p == 1:
                            emit_yproj_block(qb, last)

                    return norm

                def emit_yproj_block(qb, last):
                    for mt in range(4 * qb, 4 * qb + 4):
                        yo = ypool.tile([128, E], f32, tag="yo",
                                        name=f"yo_{mt}")
                        for nb in range(2):
                            yp = miscps.tile([128, NW], f32, tag="misc",
                                             name=f"yp_{mt}_{nb}")
                            for p2 in range(2):
                                nc.tensor.matmul(
                                    yp,
                                    at_sb[:, p2, mt * 128:(mt + 1) * 128],
                                    wo_sb[:, p2, nb * NW:(nb + 1) * NW],
                                    start=(p2 == 0), stop=(p2 == 1))
                            # at the tail the Scalar engine is free and
                            # faster to drain through than DVE
                            if last:
                                nc.scalar.copy(
                                    yo[:, nb * NW:(nb + 1) * NW], yp)
                            else:
                                nc.vector.tensor_copy(
                                    yo[:, nb * NW:(nb + 1) * NW], yp)
                        eng = nc.gpsimd if mt % 2 else nc.sync
                        eng.dma_start(y_ap[mt * 128:(mt + 1) * 128, :], yo)

                pending_norm = None
                for qb in range(NNB):
                    qsl = slice(qb * NW, (qb + 1) * NW)
                    for p in range(2):
                        pvt = [pvps.tile([65, NW], f32, tag="pv",
                                         name=f"pv_{p}_{qb}_{hh}")
                               for hh in range(2)]
                        prev = None

                        def emit_pv(kt, ptt):
                            for hh in range(2):
                                h = 2 * p + hh
                                nc.tensor.matmul(
                                    pvt[hh],
                                    v_sb[:, kt, 65 * h:65 * h + 65],
                                    ptt[:, NW * hh:NW * hh + NW],
                                    start=(kt == 0), stop=(kt == 15))

                        for kt in range(16):
                            s_ = scps.tile([128, 2 * NW], f32, tag="sc",
                                           name=f"sc_{p}_{qb}_{kt}")
                            for hh in range(2):
                                nc.tensor.matmul(
                                    s_[:, NW * hh:NW * hh + NW],
                                    kt_sb[64 * hh:64 * hh + 64, p,
                                          kt * 128:(kt + 1) * 128],
                                    qt_sb[64 * hh:64 * hh + 64, p, qsl],
                                    start=True, stop=True)
                            ptt = ptpool.tile([128, 2 * NW], DT, tag="pt",
                                              name=f"pt_{p}_{qb}_{kt}")
                            nc.scalar.activation(ptt, s_, Exp, scale=0.125)
                            if kt == 1 and pending_norm is not None:
                                pending_norm()
                                pending_norm = None
                            if prev is not None:
                                emit_pv(*prev)
                            prev = (kt, ptt)
                        emit_pv(*prev)
                        pending_norm = make_norm(
                            p, qb, pvt, last=(qb == NNB - 1 and p == 1))
                pending_norm()

    nc.compile()
    return nc


def _get_program(mode):
    if mode not in _PROGRAMS:
        _PROGRAMS[mode] = _build(mode)
    return _PROGRAMS[mode]


def kernel(q, k, v, mask, Wq, bq, Wk, bk, Wv, bv, Wo, bo):
    global LAST_RESULT
    from concourse.bass_utils import run_bass_kernel_spmd

    mode = MODE
    nc = _get_program(mode)

    import ml_dtypes
    cdt = ml_dtypes.bfloat16

    def prep(a):
        return np.ascontiguousarray(np.asarray(a).astype(cdt))

    q = np.asarray(q); k = np.asarray(k); v = np.asarray(v)
    Wq = np.asarray(Wq); Wk = np.asarray(Wk); Wv = np.asarray(Wv)
    Wo = np.asarray(Wo)
    bq = np.asarray(bq); bk = np.asarray(bk); bv = np.asarray(bv)
    bo = np.asarray(bo)

    xT = [[prep(q[b].T), prep(k[b].T), prep(v[b].T)] for b in range(B)]

    in_maps = []
    for core in range(NCORES):
        b, g = core // 4, core % 4
        r0 = g * FPC

        def wqk_layout(W):
            # lhsT tiles: [part p, ktile, m] = W.T[kt*128+p, m]
            A = W[r0:r0 + FPC, :].T.reshape(8, 128, FPC)
            return prep(A.transpose(1, 0, 2))

        WvT = Wv[r0:r0 + FPC, :].T  # (E, 256)
        Wv_aug = np.zeros((E, VW), np.float32)
        bv_aug = np.zeros((1, VW), np.float32)
        for h in range(4):
            Wv_aug[:, 65 * h:65 * h + 64] = WvT[:, 64 * h:64 * h + 64]
            bv_aug[0, 65 * h:65 * h + 64] = bv[r0 + 64 * h:r0 + 64 * h + 64]
            bv_aug[0, 65 * h + 64] = 1.0
        Wo_l = Wo[:, r0:r0 + FPC].T.reshape(2, 128, E).transpose(1, 0, 2)

        in_maps.append({
            "xq": xT[b][0], "xk": xT[b][1], "xv": xT[b][2],
            "wq": wqk_layout(Wq),
            "wk": wqk_layout(Wk),
            "wv": prep(Wv_aug.reshape(8, 128, VW).transpose(1, 0, 2)),
            "wo": prep(Wo_l),
            "bqk": np.stack([bq[r0:r0 + 128], bq[r0 + 128:r0 + FPC],
                             bk[r0:r0 + 128], bk[r0 + 128:r0 + FPC]],
                            axis=1).astype(np.float32),
            "bv": prep(bv_aug),
            "ones": np.ones((1, 128), cdt),
        })

    kwargs = {}
    if TRACE:
        kwargs = {"trace": True, "tmpdir": TRACE_DIR}
    res = run_bass_kernel_spmd(nc, in_maps, list(range(NCORES)), **kwargs)
    LAST_RESULT = res

    y = np.zeros((B, S, E), np.float32)
    for core in range(NCORES):
        y[core // 4] += res.results[core]["y"]
    y += bo.astype(np.float32)
    return y


# revision 8
# speedup vs baseline: 1.1343x; 1.1343x over previous
"""Multi-head attention (B=2, S=2048, E=1024, H=16) on 8 Trainium2 cores.

Sharding: core c -> (batch b = c//4, head-group g = c%4 of 4 heads).
Each core computes Q/K/V projections for its 4 heads (256 features),
full attention for those heads, and a partial output projection
(256 rows of Wo). Host sums the 4 partials per batch element and adds bo.

Schedule (v3): built for engine overlap.
  - x DMAs ride TWO queues (sync+scalar, alternating feature tiles) in
    priority order xk, xq, xv; weights ride the gpsimd queue. y output
    rides sync/gpsimd (NOT scalar: the Scalar engine is the exp
    bottleneck and DMA triggers would steal ~0.7us each from it).
  - Lead-in: K-proj (both pairs) tracks xk arrival f-inner on 8 psum
    accumulators, then Q-proj (both pairs) tracks xq, then V-proj in two
    8-bank passes tracks xv. Attention starts as soon as Q/K are done;
    the first query-block's P@V chases the arriving V tiles through a
    deep ptt run-ahead buffer.
  - Attention is software-pipelined one key-tile deep: PE emits
    scores(kt), then PV(kt-1), so the exp(kt) on the Scalar engine
    overlaps PE work instead of stalling it (which also lets the PE
    p-state ramp to 2.4 GHz).
  - The loop is query-block-major (qb outer, pair inner) so the output
    projection (which needs both pairs' attention output for a token
    range) spreads uniformly over the whole phase instead of piling
    into pair 1.
  - Each block's normalization chain is DEFERRED into the next block's
    kt loop so the in-order PE never waits on the DVE denominator copy.

On-chip layouts (per core):
  qt/kt: (128 feat-part, pair, 2048 tok)  transposed proj outputs; the
         128 partitions hold two heads (64+64) per pair index.
  v:     (128 tok-part, 16 tok-tiles, 4*65): per head 64 dims plus a
         "ones" column produced by an augmented V projection (extra
         output feature with zero weights and bias 1.0); P @ V_aug then
         also yields the softmax denominator row for free.
  scores are computed transposed (key-pos on partitions, query on free)
  so exp runs on ACT along the free dim and P tiles feed P@V directly as
  the moving operand; no transposes anywhere in the pipeline.
"""

import numpy as np

B, S, E, H = 2, 2048, 1024, 16
D = 64
NCORES = 8
FPC = 256  # features (head dims) per core = 4 heads
VW = 4 * 65  # V-projection output width incl. ones columns

MODE = "bf16"

_PROGRAMS = {}
LAST_RESULT = None
TRACE = False
TRACE_DIR = None


def _build(mode):
    import concourse.tile as tile
    from concourse import bacc, mybir

    f32 = mybir.dt.float32
    DT = mybir.dt.bfloat16
    NW = 512
    NNB = S // NW  # 4 query blocks per pair

    nc = bacc.Bacc("TRN2", target_bir_lowering=False, debug=False,
                   num_devices=NCORES)

    xq_ap = nc.dram_tensor("xq", [E, S], DT, kind="ExternalInput").ap()
    xk_ap = nc.dram_tensor("xk", [E, S], DT, kind="ExternalInput").ap()
    xv_ap = nc.dram_tensor("xv", [E, S], DT, kind="ExternalInput").ap()
    wq_ap = nc.dram_tensor("wq", [128, 8, FPC], DT, kind="ExternalInput").ap()
    wk_ap = nc.dram_tensor("wk", [128, 8, FPC], DT, kind="ExternalInput").ap()
    wv_ap = nc.dram_tensor("wv", [128, 8, VW], DT, kind="ExternalInput").ap()
    wo_ap = nc.dram_tensor("wo", [128, 2, E], DT, kind="ExternalInput").ap()
    bqk_ap = nc.dram_tensor("bqk", [128, 4], f32, kind="ExternalInput").ap()
    bv_ap = nc.dram_tensor("bv", [1, VW], DT, kind="ExternalInput").ap()
    ones_ap = nc.dram_tensor("ones", [1, 128], DT, kind="ExternalInput").ap()
    y_ap = nc.dram_tensor("y", [S, E], f32, kind="ExternalOutput").ap()

    Exp = mybir.ActivationFunctionType.Exp

    with tile.TileContext(nc) as tc:
        with tc.tile_pool(name="persist", bufs=1) as persist:
            wq_sb = persist.tile([128, 8, FPC], DT, name="wq_sb")
            wk_sb = persist.tile([128, 8, FPC], DT, name="wk_sb")
            wv_sb = persist.tile([128, 8, VW], DT, name="wv_sb")
            wo_sb = persist.tile([128, 2, E], DT, name="wo_sb")
            bqk_sb = persist.tile([128, 4], f32, name="bqk_sb")
            bv_sb = persist.tile([1, VW], DT, name="bv_sb")
            ones_sb = persist.tile([1, 128], DT, name="ones_sb")
            qt_sb = persist.tile([128, 2, S], DT, name="qt_sb")
            kt_sb = persist.tile([128, 2, S], DT, name="kt_sb")
            v_sb = persist.tile([128, 16, VW], DT, name="v_sb")
            at_sb = persist.tile([128, 2, S], DT, name="at_sb")
            xq_res = persist.tile([128, 8, S], DT, name="xq_res")
            xk_res = persist.tile([128, 8, S], DT, name="xk_res")
            xv_res = persist.tile([128, 8, S], DT, name="xv_res")
            ones32 = ones_sb[:, 0:64]

            # weights/consts on the gpsimd DGE queue, q first (needed first)
            nc.gpsimd.dma_start(wq_sb, wq_ap)
            nc.gpsimd.dma_start(bqk_sb, bqk_ap)
            nc.gpsimd.dma_start(wk_sb, wk_ap)
            nc.gpsimd.dma_start(wv_sb, wv_ap)
            nc.gpsimd.dma_start(bv_sb, bv_ap)
            nc.gpsimd.dma_start(ones_sb, ones_ap)
            nc.gpsimd.dma_start(wo_sb, wo_ap)
            # x on TWO queues, alternating tiles, priority xq > xk > xv:
            # attention can start once Q/K projections land; V tiles are
            # consumed one per kt unit so xv may trail (the ptt run-ahead
            # buffer bridges the gap).
            for xres, xap in ((xq_res, xq_ap), (xk_res, xk_ap),
                              (xv_res, xv_ap)):
                for f in range(8):
                    eng = nc.sync if f % 2 == 0 else nc.scalar
                    eng.dma_start(xres[:, f, :],
                                  xap[f * 128:(f + 1) * 128, :])

            def emit_qk_proj(pool, w_sb, x_res, out_sb, bcol):
                """f-inner projection, both pairs: 8 psum accumulators
                track the x feature tiles as they arrive."""
                pj = {}
                for p in range(2):
                    for nb in range(NNB):
                        pj[(p, nb)] = pool.tile(
                            [128, NW], f32, tag="proj", bufs=8,
                            name=f"pj_{bcol}_{p}_{nb}")
                for f in range(8):
                    for p in range(2):
                        for nb in range(NNB):
                            nc.tensor.matmul(
                                pj[(p, nb)],
                                w_sb[:, f, p * 128:(p + 1) * 128],
                                x_res[:, f, nb * NW:(nb + 1) * NW],
                                start=(f == 0), stop=(f == 7))
                for p in range(2):
                    for nb in range(NNB):
                        nc.vector.tensor_scalar_add(
                            out_sb[:, p, nb * NW:(nb + 1) * NW], pj[(p, nb)],
                            bqk_sb[:, bcol + p:bcol + p + 1])

            def emit_v_half(pool, half):
                """V projection for 8 token tiles, f-inner (tracks xv)."""
                psv = [pool.tile([128, VW], f32, tag="proj", bufs=8,
                                 name=f"pjv_{half}_{i}")
                       for i in range(8)]
                for i in range(8):
                    nc.tensor.matmul(psv[i], ones_sb, bv_sb,
                                     start=True, stop=False)
                for f in range(8):
                    for i in range(8):
                        tt = half * 8 + i
                        nc.tensor.matmul(
                            psv[i],
                            xv_res[:, f, tt * 128:(tt + 1) * 128],
                            wv_sb[:, f, :],
                            start=False, stop=(f == 7))
                for i in range(8):
                    nc.vector.tensor_copy(v_sb[:, half * 8 + i, :], psv[i])

            # ---- lead-in: Q-proj, K-proj (both pairs), V-proj ----
            with tc.tile_pool(name="lead", bufs=1, space="PSUM") as lead:
                emit_qk_proj(lead, wq_sb, xq_res, qt_sb, 0)
                emit_qk_proj(lead, wk_sb, xk_res, kt_sb, 2)
                emit_v_half(lead, 0)
                emit_v_half(lead, 1)

            # ---- attention: flat unit stream, 2-deep PV skew ----
            # Units are (qb, p, kt); the PE emits scores(u_i) then
            # PV(u_{i-2}) so exp latency on the Scalar engine never blocks
            # the in-order PE queue — including across block boundaries.
            # Each block's normalization chain is deferred into the next
            # block's kt==1 slot.
            with tc.tile_pool(name="pt", bufs=14) as ptpool, \
                 tc.tile_pool(name="sm", bufs=2) as smpool, \
                 tc.tile_pool(name="ysb", bufs=2) as ypool, \
                 tc.tile_pool(name="scps", bufs=2, space="PSUM") as scps, \
                 tc.tile_pool(name="pvps", bufs=3, space="PSUM") as pvps, \
                 tc.tile_pool(name="miscps", bufs=1, space="PSUM") as miscps:

                pvt_of = {}  # (qb, p) -> [h0_tile, h1_tile]

                def emit_yproj_block(qb, last):
                    for mt in range(4 * qb, 4 * qb + 4):
                        yo = ypool.tile([128, E], f32, tag="yo",
                                        name=f"yo_{mt}")
                        for nb in range(2):
                            yp = miscps.tile([128, NW], f32, tag="misc",
                                             name=f"yp_{mt}_{nb}")
                            for p2 in range(2):
                                nc.tensor.matmul(
                                    yp,
                                    at_sb[:, p2, mt * 128:(mt + 1) * 128],
                                    wo_sb[:, p2, nb * NW:(nb + 1) * NW],
                                    start=(p2 == 0), stop=(p2 == 1))
                            # gpsimd cannot read PSUM; drain via DVE, and
                            # via the (by then idle) Scalar engine at the
                            # tail
                            if last:
                                nc.scalar.copy(
                                    yo[:, nb * NW:(nb + 1) * NW], yp)
                            else:
                                nc.vector.tensor_copy(
                                    yo[:, nb * NW:(nb + 1) * NW], yp)
                        eng = nc.gpsimd if mt % 2 else nc.sync
                        eng.dma_start(y_ap[mt * 128:(mt + 1) * 128, :], yo)

                def emit_norm(qb, p, last=False):
                    qsl = slice(qb * NW, (qb + 1) * NW)
                    pvt = pvt_of.pop((qb, p))
                    for hh in range(2):
                        denr = smpool.tile([1, NW], DT, tag="denr",
                                           name=f"dn_{p}_{qb}_{hh}")
                        nc.vector.tensor_copy(denr, pvt[hh][64:65, :])
                        rb = miscps.tile([64, NW], f32, tag="misc",
                                         name=f"rb_{p}_{qb}_{hh}")
                        nc.tensor.matmul(rb, ones32, denr,
                                         start=True, stop=True)
                        rbs = smpool.tile([64, NW], f32, tag="rbs",
                                          name=f"rbs_{p}_{qb}_{hh}")
                        nc.vector.reciprocal_approx_fast(rbs, rb)
                        nc.vector.tensor_mul(
                            at_sb[64 * hh:64 * hh + 64, p, qsl],
                            pvt[hh][0:64, :], rbs)
                    if p == 1:
                        emit_yproj_block(qb, last)

                units = [(qb, p, kt)
                         for qb in range(NNB) for p in range(2)
                         for kt in range(16)]
                pend = []  # pv emissions in flight (skew 2)

                def emit_pv(qb, p, kt, ptt):
                    if kt == 0:
                        pvt_of[(qb, p)] = [
                            pvps.tile([65, NW], f32, tag="pv",
                                      name=f"pv_{p}_{qb}_{hh}")
                            for hh in range(2)]
                    pvt = pvt_of[(qb, p)]
                    for hh in range(2):
                        h = 2 * p + hh
                        nc.tensor.matmul(
                            pvt[hh],
                            v_sb[:, kt, 65 * h:65 * h + 65],
                            ptt[:, NW * hh:NW * hh + NW],
                            start=(kt == 0), stop=(kt == 15))

                for qb, p, kt in units:
                    qsl = slice(qb * NW, (qb + 1) * NW)
                    s_ = scps.tile([128, 2 * NW], f32, tag="sc",
                                   name=f"sc_{p}_{qb}_{kt}")
                    for hh in range(2):
                        nc.tensor.matmul(
                            s_[:, NW * hh:NW * hh + NW],
                            kt_sb[64 * hh:64 * hh + 64, p,
                                  kt * 128:(kt + 1) * 128],
                            qt_sb[64 * hh:64 * hh + 64, p, qsl],
                            start=True, stop=True)
                    ptt = ptpool.tile([128, 2 * NW], DT, tag="pt",
                                      name=f"pt_{p}_{qb}_{kt}")
                    nc.scalar.activation(ptt, s_, Exp, scale=0.125)
                    pend.append((qb, p, kt, ptt))
                    if kt == 3 and (qb, p) != (0, 0):
                        # previous block's pv(15) was emitted one unit ago;
                        # normalize it now, BEFORE this block's pv(0) below
                        # (whose pvt slot rotation waits on this at-mul).
                        pqb, pp = (qb, p - 1) if p == 1 else (qb - 1, 1)
                        emit_norm(pqb, pp)
                    if len(pend) > 3:
                        emit_pv(*pend.pop(0))
                while pend:
                    emit_pv(*pend.pop(0))
                emit_norm(NNB - 1, 1, last=True)

    nc.compile()
    return nc


def _get_program(mode):
    if mode not in _PROGRAMS:
        _PROGRAMS[mode] = _build(mode)
    return _PROGRAMS[mode]


def kernel(q, k, v, mask, Wq, bq, Wk, bk, Wv, bv, Wo, bo):
    global LAST_RESULT
    from concourse.bass_utils import run_bass_kernel_spmd

    mode = MODE
    nc = _get_program(mode)

    import ml_dtypes
    cdt = ml_dtypes.bfloat16

    def prep(a):
        return np.ascontiguousarray(np.asarray(a).astype(cdt))

    q = np.asarray(q); k = np.asarray(k); v = np.asarray(v)
    Wq = np.asarray(Wq); Wk = np.asarray(Wk); Wv = np.asarray(Wv)
    Wo = np.asarray(Wo)
    bq = np.asarray(bq); bk = np.asarray(bk); bv = np.asarray(bv)
    bo = np.asarray(bo)

    xT = [[prep(q[b].T), prep(k[b].T), prep(v[b].T)] for b in range(B)]

    in_maps = []
    for core in range(NCORES):
        b, g = core // 4, core % 4
        r0 = g * FPC

        def wqk_layout(W):
            # lhsT tiles: [part p, ktile, m] = W.T[kt*128+p, m]
            A = W[r0:r0 + FPC, :].T.reshape(8, 128, FPC)
            return prep(A.transpose(1, 0, 2))

        WvT = Wv[r0:r0 + FPC, :].T  # (E, 256)
        Wv_aug = np.zeros((E, VW), np.float32)
        bv_aug = np.zeros((1, VW), np.float32)
        for h in range(4):
            Wv_aug[:, 65 * h:65 * h + 64] = WvT[:, 64 * h:64 * h + 64]
            bv_aug[0, 65 * h:65 * h + 64] = bv[r0 + 64 * h:r0 + 64 * h + 64]
            bv_aug[0, 65 * h + 64] = 1.0
        Wo_l = Wo[:, r0:r0 + FPC].T.reshape(2, 128, E).transpose(1, 0, 2)

        in_maps.append({
            "xq": xT[b][0], "xk": xT[b][1], "xv": xT[b][2],
            "wq": wqk_layout(Wq),
            "wk": wqk_layout(Wk),
            "wv": prep(Wv_aug.reshape(8, 128, VW).transpose(1, 0, 2)),
            "wo": prep(Wo_l),
            "bqk": np.stack([bq[r0:r0 + 128], bq[r0 + 128:r0 + FPC],
                             bk[r0:r0 + 128], bk[r0 + 128:r0 + FPC]],
                            axis=1).astype(np.float32),
            "bv": prep(bv_aug),
            "ones": np.ones((1, 128), cdt),
        })

    kwargs = {}
    if TRACE:
        kwargs = {"trace": True, "tmpdir": TRACE_DIR}
    res = run_bass_kernel_spmd(nc, in_maps, list(range(NCORES)), **kwargs)
    LAST_RESULT = res

    y = np.zeros((B, S, E), np.float32)
    for core in range(NCORES):
        y[core // 4] += res.results[core]["y"]
    y += bo.astype(np.float32)
    return y
